# revision 39
# baseline (speedup 1.0000x reference)
"""BigBird block-sparse self-attention on 8 Trainium2 NeuronCores.

Reference semantics (B=4, l=4096, d=768, BLOCK=128):
  q,k,v = split(qkv); scores = q @ k^T / sqrt(d)
  mask: row i attends to j iff j<=i AND (j < 256 [global cols]
        OR j >= i-384 [sliding window] OR (i,j) in random 2x2 cells
        derived from pos0/pos1 pairs with pos0>pos1)
  out = softmax(scores + mask) @ v   (masked entries get <=-1e4, whose
        exp underflows to exactly 0 in fp32)

Strategy: 128-row blocks paired into 256-row "pair" units (N=256 matmul
free dim). 16 pairs x 4 batches = 64 units; 8 cores x 8 units each
(core c: batch c//2, pairs 8*(c%2)..8*(c%2)+7). Per unit the column
blocks are: 2 global blocks, 5 window blocks (2m-3..2m+1), and NE
gathered "extra" blocks holding the random cells not covered by
global/window. Scores are computed transposed (S^T[jj,ii]) so softmax
normalization comes out of the AV matmul via a ones-column appended to
V; masks are 0/1 multiplicative on P=exp(S^T) (underflow-equivalent to
the reference's additive -1e4). Matmuls run in bf16 with fp32 PSUM
accumulation; output is fp32.

K and V for each column block are packed into one HBM tensor so each
block loads with a single DMA (HWDGE trigger costs ~0.6us of Sync
sequencer time); output stores go through GpSimd SWDGE to keep the
Sync sequencer off the critical path.
"""

import os
import sys

for _p in ("/opt/trn_rl_repo", "/root/.axon_site/_ro/trn_rl_repo"):
    if _p not in sys.path and os.path.isdir(_p):
        sys.path.append(_p)

import ml_dtypes
import numpy as np

import concourse.bass as bass
import concourse.mybir as mybir
from bass_rust import InstNoOp
from concourse import tile as tile_mod
from concourse.bass_utils import run_bass_kernel_spmd
from concourse.tile import TileContext


def _fast_exit(self, tick_clock, wait_clock):
    """Cheaper TileContext exit than drain + EVSEM butterfly x2 (~8-10us).

    The SP drain already carries sem waits covering the global vector clock
    (= every engine's and DMA queue's final tick), so once it retires no
    engine is running or waiting. A single cross-engine sem then orders the
    GpSimd sem/dma-queue clears after it. Saves ~6us of kernel tail.
    """
    nc = self.nc
    drain_inst = nc.sync.drain()
    wait_clock.add_sem_waits(
        drain_inst.ins, tile_mod.ScopedClock({None: tick_clock.global_clock})
    )
    # split the global-clock wait chain across SP and GpSimd so the two
    # halves are checked in parallel (the post-pass later explodes each
    # side into 1-wait NoOps for walrus)
    si = drain_inst.ins.sync_info
    waits = list(si.on_wait) if (si is not None and si.on_wait) else []
    mid = len(waits) // 2
    if mid:
        drain_inst.ins.sync_info = mybir.SyncInfo(
            on_wait=waits[mid:], on_update=list(si.on_update or [])
        )
    done = nc.alloc_semaphore("tile_exit_done")
    drain_inst.then_inc(done)
    popped = nc._tile_sem_poison_stack.pop()
    assert popped is self._sem_poison
    if mid:
        gn = nc.gpsimd.nop()
        gn.ins.sync_info = mybir.SyncInfo(on_wait=waits[:mid], on_update=[])
    nc.gpsimd.wait_ge(done, 1)
    nc.clear_and_free_semaphores(list(self.sems.allocated().values()) + [done])


tile_mod.TileContext._drain_and_barrier = _fast_exit

BLOCK = 128
B, L, D = 4, 4096, 768
NB = L // BLOCK          # 32 column blocks
WINDOW = 3 * BLOCK       # 384
GLOBAL = 2 * BLOCK       # 256
NCHUNK = D // 128        # 6 contraction chunks
N_CORES = 8
UNITS = 8                # pairs per core
PAIR_ROWS = 2 * BLOCK    # 256
NWIN = 2 * UNITS + 3     # 19 window blocks cached per core
DV = D + 1               # v block width incl. ones column (769)

F32 = mybir.dt.float32
BF16 = mybir.dt.bfloat16
BF16_NP = ml_dtypes.bfloat16

# packed free-dim offsets
QM_W = NCHUNK * PAIR_ROWS + 2 * PAIR_ROWS          # qt | mg      (2048)
KVW_W = NCHUNK * 128 + DV                          # kt | v       (1537)
KVX_W = NCHUNK * 128 + DV + PAIR_ROWS              # kt | v | mx  (1793)
CR_W = 4 * PAIR_ROWS + 2 * DV                      # m14 | vg     (2562)

_last_results = None     # test harness reads exec_time_ns from here


# ---------------------------------------------------------------------------
# walrus in this toolchain rejects >1 sync-wait command per instruction;
# split excess waits onto same-engine NoOps inserted just before.
def _split_excess_waits(nc, max_w=1):
    ctr = 0
    for blk in nc.m.functions[0].blocks:
        out = []
        changed = False
        for inst in blk.instructions:
            si = inst.sync_info
            waits = list(si.on_wait) if (si is not None and si.on_wait) else []
            if len(waits) > max_w:
                changed = True
                excess, keep = waits[:-max_w], waits[-max_w:]
                for i in range(0, len(excess), max_w):
                    ctr += 1
                    nop = InstNoOp(name=f"wsplit_{ctr}", ins=[], outs=[])
                    nop.engine = inst.engine
                    nop.sync_info = mybir.SyncInfo(
                        on_wait=excess[i : i + max_w], on_update=[]
                    )
                    out.append(nop)
                inst.sync_info = mybir.SyncInfo(
                    on_wait=keep, on_update=list(si.on_update or [])
                )
            out.append(inst)
        if changed:
            blk.instructions = out


# ---------------------------------------------------------------------------
# host-side data prep

def _transpose_block(x):
    """[rows, 768] -> [128, NCHUNK*rows] with [p, c*rows+r] = x[r, c*128+p]."""
    rows = x.shape[0]
    return np.ascontiguousarray(
        x.reshape(rows, NCHUNK, 128).transpose(2, 1, 0).reshape(128, NCHUNK * rows)
    )


def _static_masks():
    ii = np.arange(PAIR_ROWS)[None, :]
    jj = np.arange(128)[:, None]
    m1 = (jj <= ii)            # block 2m   : (TRI_L | FULL)
    m2 = (jj + 128 <= ii)      # block 2m+1 : (ZERO  | TRI_L)
    m3 = (jj >= ii - 128)      # block 2m-2 : (FULL  | TRI_U)
    m4 = (jj >= ii)            # block 2m-3 : (TRI_U | ZERO)
    m5 = m4 | m2               # merged slot: left half from block 2m-3,
    #                            right half from block 2m+1 (disjoint supports)
    return [m.astype(BF16_NP) for m in (m1, m2, m3, m5)]


def _extra_cells(pos0, pos1):
    """Random-attention cells not covered by global/window, deduped.
    Returns (I, J) row/col arrays (batch-independent)."""
    pos0 = np.asarray(pos0).astype(np.int64).ravel()
    pos1 = np.asarray(pos1).astype(np.int64).ravel()
    valid = pos0 > pos1
    p0, p1 = pos0[valid], pos1[valid]
    I = np.concatenate([p0, p0, p0 + 1, p0 + 1])
    J = np.concatenate([p1, p1 + 1, p1, p1 + 1])
    ok = (J <= I) & (I < L) & (J < L)
    covered = (J < GLOBAL) | (J >= I - WINDOW)
    keep = ok & ~covered
    I, J = I[keep], J[keep]
    lin = np.unique(I * L + J)
    return lin // L, lin % L


def _prepare(qkv, pos0, pos1):
    qkv = np.asarray(qkv, dtype=np.float32)
    q = qkv[:, :, 0:D]
    k = qkv[:, :, D : 2 * D]
    v = qkv[:, :, 2 * D : 3 * D]
    scale = 1.0 / float(np.sqrt(D))

    m1, m2, m3, m5 = _static_masks()

    I, J = _extra_cells(pos0, pos1)
    pair_of = I // PAIR_ROWS
    pair_cols = {}
    for m in range(L // PAIR_ROWS):
        sel = pair_of == m
        pair_cols[m] = np.unique(J[sel])
    e_max = max((len(c) for c in pair_cols.values()), default=0)
    NE = max(1, -(-e_max // 128))                          # extra slots per unit

    # per-pair data-independent extra masks [NE,128,256]
    pair_mx = {}
    for m, cols in pair_cols.items():
        mx = np.zeros((NE, 128, PAIR_ROWS), dtype=BF16_NP)
        if len(cols):
            sel = pair_of == m
            e = np.searchsorted(cols, J[sel])
            mx[e // 128, e % 128, I[sel] - m * PAIR_ROWS] = 1.0
        pair_mx[m] = mx

    in_maps = []
    for c in range(N_CORES):
        b, h = c // 2, c % 2
        kb, vb, qb = k[b], v[b], q[b]

        kvw = np.zeros((NWIN, 128, KVW_W), dtype=BF16_NP)
        for j in range(NWIN):
            blk = 16 * h - 3 + j
            if 2 <= blk < NB:                 # blocks 0,1 are served by the
                r0 = blk * BLOCK              # global slots; <0 don't exist
                kvw[j, :, : NCHUNK * 128] = _transpose_block(kb[r0 : r0 + BLOCK])
                kvw[j, :, NCHUNK * 128 : NCHUNK * 128 + D] = vb[r0 : r0 + BLOCK]
                kvw[j, :, NCHUNK * 128 + D] = 1.0

        ktg = np.concatenate(
            [_transpose_block(kb[0:BLOCK]), _transpose_block(kb[BLOCK:GLOBAL])], axis=1
        ).astype(BF16_NP)                                  # [128, 2*768]

        crest = np.zeros((128, CR_W), dtype=BF16_NP)       # m14 | vg
        for i, m in enumerate((m1, m2, m3, m5)):
            crest[:, i * PAIR_ROWS : (i + 1) * PAIR_ROWS] = m
        o = 4 * PAIR_ROWS
        crest[:, o : o + D] = vb[0:BLOCK]
        crest[:, o + D] = 1.0
        crest[:, o + DV : o + DV + D] = vb[BLOCK:GLOBAL]
        crest[:, o + DV + D] = 1.0

        qm = np.zeros((UNITS, 128, QM_W), dtype=BF16_NP)   # qt | mg
        kvx = np.zeros((UNITS * NE, 128, KVX_W), dtype=BF16_NP)
        for u in range(UNITS):
            m = 8 * h + u
            r0 = m * PAIR_ROWS
            qm[u, :, : NCHUNK * PAIR_ROWS] = _transpose_block(
                qb[r0 : r0 + PAIR_ROWS] * scale
            )
            og = NCHUNK * PAIR_ROWS
            qm[u, :, og:] = 1.0
            if m == 0:  # pair (0,1): global cols need the causal triangle
                qm[u, :, og : og + PAIR_ROWS] = m1
                qm[u, :, og + PAIR_ROWS :] = m2
            cols = pair_cols[m]
            for x in range(NE):
                cc = cols[x * 128 : (x + 1) * 128]
                row = kvx[u * NE + x]
                if len(cc):
                    kx = np.zeros((128, D), dtype=np.float32)
                    kx[: len(cc)] = kb[cc]
                    row[:, : NCHUNK * 128] = _transpose_block(kx)
                    row[: len(cc), NCHUNK * 128 : NCHUNK * 128 + D] = vb[cc]
                    row[: len(cc), NCHUNK * 128 + D] = 1.0
                row[:, NCHUNK * 128 + DV :] = pair_mx[m][x]

        in_maps.append({"qm": qm, "kvw": kvw, "ktg": ktg, "crest": crest, "kvx": kvx})
    return in_maps, NE


# ---------------------------------------------------------------------------
# device program (identical across cores; all variation is in the data)

def _build_program(NE):
    nc = bass.Bass()
    d_qm = nc.dram_tensor("qm", [UNITS, 128, QM_W], BF16, kind="ExternalInput")
    d_kvw = nc.dram_tensor("kvw", [NWIN, 128, KVW_W], BF16, kind="ExternalInput")
    d_ktg = nc.dram_tensor("ktg", [128, 2 * NCHUNK * 128], BF16, kind="ExternalInput")
    d_crest = nc.dram_tensor("crest", [128, CR_W], BF16, kind="ExternalInput")
    d_kvx = nc.dram_tensor("kvx", [UNITS * NE, 128, KVX_W], BF16, kind="ExternalInput")
    d_out = nc.dram_tensor("out", [UNITS, PAIR_ROWS, D], F32, kind="ExternalOutput")

    EXP = mybir.ActivationFunctionType.Exp
    KTW = NCHUNK * 128   # 768: kt width inside kvw/kvx rows

    with TileContext(nc) as tc:
        with (
            tc.tile_pool(name="const", bufs=1) as const_pool,
            tc.tile_pool(name="kvw", bufs=8) as kvw_pool,
            tc.tile_pool(name="qm", bufs=2) as qm_pool,
            tc.tile_pool(name="ext", bufs=NE + 1) as ext_pool,
            tc.tile_pool(name="pt", bufs=16) as pt_pool,
            tc.tile_pool(name="eps", bufs=4) as eps_pool,
            tc.tile_pool(name="ob", bufs=3) as ob_pool,
            tc.tile_pool(name="st", bufs=4, space="PSUM") as st_pool,
            tc.tile_pool(name="av", bufs=2, space="PSUM") as av_pool,
        ):
            # No PE warm-up block: the HAM cold-clock tax (~3.4us at 1.2GHz)
            # applies to the first matmuls either way, so spend it on REAL
            # work — slot 0 starts as soon as its half of ktg + qt land.
            ktg_t = const_pool.tile([128, 2 * NCHUNK * 128], BF16)
            nc.sync.dma_start(
                out=ktg_t[:, : NCHUNK * 128], in_=d_ktg[:, : NCHUNK * 128]
            )
            # split qt/mg halves so the first QK only waits on the qt bytes
            qm_first = qm_pool.tile([128, QM_W], BF16, tag="qm", name="qm0")
            _oq = NCHUNK * PAIR_ROWS
            nc.sync.dma_start(out=qm_first[:, :_oq], in_=d_qm[0, :, :_oq])
            nc.sync.dma_start(
                out=ktg_t[:, NCHUNK * 128 :], in_=d_ktg[:, NCHUNK * 128 :]
            )
            nc.sync.dma_start(out=qm_first[:, _oq:], in_=d_qm[0, :, _oq:])

            kvw_tiles = {}

            def load_kvw(j):
                if j not in kvw_tiles:
                    t = kvw_pool.tile([128, KVW_W], BF16, tag="kvw", name=f"kvw{j}")
                    nc.sync.dma_start(out=t[:], in_=d_kvw[j])
                    kvw_tiles[j] = t

            load_kvw(1)
            load_kvw(2)
            crest_t = const_pool.tile([128, CR_W], BF16)
            nc.sync.dma_start(out=crest_t[:], in_=d_crest[:])

            def m14_ap(idx):
                return crest_t[:, idx * PAIR_ROWS : (idx + 1) * PAIR_ROWS]

            def vg_ap(idx):
                o = 4 * PAIR_ROWS + idx * DV
                return crest_t[:, o : o + DV]

            for u in range(UNITS):
                if u == 0:
                    qm_t = qm_first
                else:
                    qm_t = qm_pool.tile([128, QM_W], BF16, tag="qm", name=f"qm{u}")
                    nc.sync.dma_start(out=qm_t[:], in_=d_qm[u])
                qt = qm_t[:, : NCHUNK * PAIR_ROWS]

                def mg_ap(idx):
                    o = NCHUNK * PAIR_ROWS + idx * PAIR_ROWS
                    return qm_t[:, o : o + PAIR_ROWS]

                # slot order consumes cached tiles first; new tiles (2u+3,
                # 2u+4) load in the order the reordered slots need them
                for j in (2 * u + 1, 2 * u + 2, 2 * u + 3, 2 * u, 2 * u + 4):
                    load_kvw(j)
                kvx_ts = []
                for x in range(NE):
                    t = ext_pool.tile([128, KVX_W], BF16, tag="kvx", name=f"kvx{u}_{x}")
                    nc.sync.dma_start(out=t[:], in_=d_kvx[u * NE + x])
                    kvx_ts.append(t)

                def wkt(j):
                    return kvw_tiles[j][:, 0:KTW]

                def wv(j):
                    return kvw_tiles[j][:, KTW : KTW + DV]

                # slot list: (kT per half (same AP or two APs), V per half,
                # mask access or None). The merged slot computes the left
                # score half from block 2m-3 and the right half from block
                # 2m+1 (their masked supports are complementary halves).
                # mg is all-ones except pair 0 (h=0 cores' unit 0), whose
                # global columns need the causal triangle — all cores' unit 0
                # carries the multiply (h=1 data is ones), later units skip it
                slots = [
                    ((ktg_t[:, 0:KTW],) * 2, (vg_ap(0),) * 2,
                     mg_ap(0) if u == 0 else None),
                    ((ktg_t[:, KTW:],) * 2, (vg_ap(1),) * 2,
                     mg_ap(1) if u == 0 else None),
                    ((wkt(2 * u + 1),) * 2, (wv(2 * u + 1),) * 2, m14_ap(2)),  # M3
                    ((wkt(2 * u + 2),) * 2, (wv(2 * u + 2),) * 2, None),       # full
                    ((wkt(2 * u + 3),) * 2, (wv(2 * u + 3),) * 2, m14_ap(0)),  # M1
                    (
                        (wkt(2 * u), wkt(2 * u + 4)),
                        (wv(2 * u), wv(2 * u + 4)),
                        m14_ap(3),                           # M5 merged
                    ),
                ] + [
                    (
                        (kvx_ts[x][:, 0:KTW],) * 2,
                        (kvx_ts[x][:, KTW : KTW + DV],) * 2,
                        kvx_ts[x][:, KTW + DV :],
                    )
                    for x in range(NE)
                ]

                av = [
                    av_pool.tile([128, DV], F32, tag="av", name=f"av{u}_{hh}")
                    for hh in range(2)
                ]
                n_s = len(slots)

                def emit_av(si, pt, v_aps):
                    for hh in range(2):
                        lhs = pt[:, hh * 128 : (hh + 1) * 128]
                        v_ap = v_aps[hh]
                        nc.tensor.matmul(
                            av[hh][:, 0:512], lhs, v_ap[:, 0:512],
                            start=(si == 0), stop=(si == n_s - 1),
                        )
                        nc.tensor.matmul(
                            av[hh][:, 512:DV], lhs, v_ap[:, 512:DV],
                            start=(si == 0), stop=(si == n_s - 1),
                        )

                # AV for slot si is emitted after QK for slot si+SKEW so the
                # PE has guaranteed work while exp+mask of slot si complete.
                pending = []
                for si, (kt_aps, v_aps, m_ap) in enumerate(slots):
                    st = st_pool.tile([128, PAIR_ROWS], F32, tag="st", name=f"st{u}_{si}")
                    if kt_aps[0] is kt_aps[1]:
                        for cc in range(NCHUNK):
                            nc.tensor.matmul(
                                st[:],
                                kt_aps[0][:, cc * 128 : (cc + 1) * 128],
                                qt[:, cc * PAIR_ROWS : (cc + 1) * PAIR_ROWS],
                                start=(cc == 0),
                                stop=(cc == NCHUNK - 1),
                            )
                    else:
                        # two sequential accumulation groups in one PSUM bank
                        # (zero region = bank, so they must not interleave)
                        for hh in range(2):
                            for cc in range(NCHUNK):
                                nc.tensor.matmul(
                                    st[:, hh * 128 : (hh + 1) * 128],
                                    kt_aps[hh][:, cc * 128 : (cc + 1) * 128],
                                    qt[:, cc * PAIR_ROWS + hh * 128 : cc * PAIR_ROWS + hh * 128 + 128],
                                    start=(cc == 0),
                                    stop=(cc == NCHUNK - 1),
                                )
                    pt = pt_pool.tile([128, PAIR_ROWS], BF16, tag="pt", name=f"pt{u}_{si}")
                    nc.scalar.activation(pt[:], st[:], EXP)
                    if m_ap is not None:
                        nc.vector.tensor_mul(pt[:], pt[:], m_ap)
                    pending.append((si, pt, v_aps))
                    if len(pending) > 2:
                        emit_av(*pending.pop(0))
                for p in pending:
                    emit_av(*p)
                for hh in range(2):
                    rc = eps_pool.tile([128, 1], F32, tag="rc", name=f"rc{u}_{hh}")
                    nc.vector.reciprocal(rc[:], av[hh][:, D : D + 1])
                    ob = ob_pool.tile([128, D], F32, tag="ob", name=f"ob{u}_{hh}")
                    # chunked scale+store, spread across engines (scale on
                    # ACT for half 0 / DVE for half 1; store triggers on
                    # GpSimd / Sync) so the kernel tail pipelines
                    for lo, hi in ((0, 384), (384, D)):
                        if hh == 0:
                            nc.scalar.mul(ob[:, lo:hi], av[hh][:, lo:hi], rc[:])
                            st_eng = nc.gpsimd
                        else:
                            nc.vector.tensor_scalar_mul(
                                ob[:, lo:hi], av[hh][:, lo:hi], rc[:]
                            )
                            st_eng = nc.sync
                        st_eng.dma_start(
                            out=d_out[u, hh * 128 : (hh + 1) * 128, lo:hi],
                            in_=ob[:, lo:hi],
                        )

    _split_excess_waits(nc, max_w=1)
    return nc


_program_cache = {}


def kernel(qkv, pos0, pos1):
    global _last_results
    in_maps, NE = _prepare(qkv, pos0, pos1)
    if NE not in _program_cache:
        _program_cache[NE] = _build_program(NE)
    nc = _program_cache[NE]
    res = run_bass_kernel_spmd(
        nc, in_maps, core_ids=list(range(N_CORES)),
        trace=bool(os.environ.get("BASS_TRACE")),
    )
    _last_results = res
    out = np.empty((B, L, D), dtype=np.float32)
    for c in range(N_CORES):
        b, h = c // 2, c % 2
        for u in range(UNITS):
            r0 = (8 * h + u) * PAIR_ROWS
            out[b, r0 : r0 + PAIR_ROWS, :] = res.results[c]["out"][u]
    return out


# revision 41
# speedup vs baseline: 1.0103x; 1.0103x over previous
"""BigBird block-sparse self-attention on 8 Trainium2 NeuronCores.

Reference semantics (B=4, l=4096, d=768, BLOCK=128):
  q,k,v = split(qkv); scores = q @ k^T / sqrt(d)
  mask: row i attends to j iff j<=i AND (j < 256 [global cols]
        OR j >= i-384 [sliding window] OR (i,j) in random 2x2 cells
        derived from pos0/pos1 pairs with pos0>pos1)
  out = softmax(scores + mask) @ v   (masked entries get <=-1e4, whose
        exp underflows to exactly 0 in fp32)

Strategy: 128-row blocks paired into 256-row "pair" units (N=256 matmul
free dim). 16 pairs x 4 batches = 64 units; 8 cores x 8 units each
(core c: batch c//2, pairs 8*(c%2)..8*(c%2)+7). Per unit the column
blocks are: 2 global blocks, 5 window blocks (2m-3..2m+1), and NE
gathered "extra" blocks holding the random cells not covered by
global/window. Scores are computed transposed (S^T[jj,ii]) so softmax
normalization comes out of the AV matmul via a ones-column appended to
V; masks are 0/1 multiplicative on P=exp(S^T) (underflow-equivalent to
the reference's additive -1e4). Matmuls run in bf16 with fp32 PSUM
accumulation; output is fp32.

K and V for each column block are packed into one HBM tensor so each
block loads with a single DMA (HWDGE trigger costs ~0.6us of Sync
sequencer time); output stores go through GpSimd SWDGE to keep the
Sync sequencer off the critical path.
"""

import os
import sys

for _p in ("/opt/trn_rl_repo", "/root/.axon_site/_ro/trn_rl_repo"):
    if _p not in sys.path and os.path.isdir(_p):
        sys.path.append(_p)

import ml_dtypes
import numpy as np

import concourse.bass as bass
import concourse.mybir as mybir
from bass_rust import InstNoOp
from concourse import tile as tile_mod
from concourse.bass_utils import run_bass_kernel_spmd
from concourse.tile import TileContext


def _fast_exit(self, tick_clock, wait_clock):
    """Cheaper TileContext exit than drain + EVSEM butterfly x2 (~8-10us).

    The SP drain already carries sem waits covering the global vector clock
    (= every engine's and DMA queue's final tick), so once it retires no
    engine is running or waiting. A single cross-engine sem then orders the
    GpSimd sem/dma-queue clears after it. Saves ~6us of kernel tail.
    """
    nc = self.nc
    drain_inst = nc.sync.drain()
    wait_clock.add_sem_waits(
        drain_inst.ins, tile_mod.ScopedClock({None: tick_clock.global_clock})
    )
    # split the global-clock wait chain across SP and GpSimd so the two
    # halves are checked in parallel (the post-pass later explodes each
    # side into 1-wait NoOps for walrus)
    si = drain_inst.ins.sync_info
    waits = list(si.on_wait) if (si is not None and si.on_wait) else []
    mid = len(waits) // 2
    if mid:
        drain_inst.ins.sync_info = mybir.SyncInfo(
            on_wait=waits[mid:], on_update=list(si.on_update or [])
        )
    done = nc.alloc_semaphore("tile_exit_done")
    drain_inst.then_inc(done)
    popped = nc._tile_sem_poison_stack.pop()
    assert popped is self._sem_poison
    if mid:
        gn = nc.gpsimd.nop()
        gn.ins.sync_info = mybir.SyncInfo(on_wait=waits[:mid], on_update=[])
    nc.gpsimd.wait_ge(done, 1)
    nc.clear_and_free_semaphores(list(self.sems.allocated().values()) + [done])


tile_mod.TileContext._drain_and_barrier = _fast_exit

BLOCK = 128
B, L, D = 4, 4096, 768
NB = L // BLOCK          # 32 column blocks
WINDOW = 3 * BLOCK       # 384
GLOBAL = 2 * BLOCK       # 256
NCHUNK = D // 128        # 6 contraction chunks
N_CORES = 8
UNITS = 8                # pairs per core
PAIR_ROWS = 2 * BLOCK    # 256
NWIN = 2 * UNITS + 3     # 19 window blocks cached per core
DV = D + 1               # v block width incl. ones column (769)

F32 = mybir.dt.float32
BF16 = mybir.dt.bfloat16
BF16_NP = ml_dtypes.bfloat16

# packed free-dim offsets
QM_W = NCHUNK * PAIR_ROWS + 2 * PAIR_ROWS          # qt | mg      (2048)
KVW_W = NCHUNK * 128 + DV                          # kt | v       (1537)
KVX_W = NCHUNK * 128 + DV + PAIR_ROWS              # kt | v | mx  (1793)
CR_W = 4 * PAIR_ROWS + 2 * DV                      # m14 | vg     (2562)

_last_results = None     # test harness reads exec_time_ns from here


# ---------------------------------------------------------------------------
# walrus in this toolchain rejects >1 sync-wait command per instruction;
# split excess waits onto same-engine NoOps inserted just before.
def _split_excess_waits(nc, max_w=1):
    ctr = 0
    for blk in nc.m.functions[0].blocks:
        out = []
        changed = False
        for inst in blk.instructions:
            si = inst.sync_info
            waits = list(si.on_wait) if (si is not None and si.on_wait) else []
            if len(waits) > max_w:
                changed = True
                excess, keep = waits[:-max_w], waits[-max_w:]
                for i in range(0, len(excess), max_w):
                    ctr += 1
                    nop = InstNoOp(name=f"wsplit_{ctr}", ins=[], outs=[])
                    nop.engine = inst.engine
                    nop.sync_info = mybir.SyncInfo(
                        on_wait=excess[i : i + max_w], on_update=[]
                    )
                    out.append(nop)
                inst.sync_info = mybir.SyncInfo(
                    on_wait=keep, on_update=list(si.on_update or [])
                )
            out.append(inst)
        if changed:
            blk.instructions = out


# ---------------------------------------------------------------------------
# host-side data prep

def _transpose_block(x):
    """[rows, 768] -> [128, NCHUNK*rows] with [p, c*rows+r] = x[r, c*128+p]."""
    rows = x.shape[0]
    return np.ascontiguousarray(
        x.reshape(rows, NCHUNK, 128).transpose(2, 1, 0).reshape(128, NCHUNK * rows)
    )


def _static_masks():
    ii = np.arange(PAIR_ROWS)[None, :]
    jj = np.arange(128)[:, None]
    m1 = (jj <= ii)            # block 2m   : (TRI_L | FULL)
    m2 = (jj + 128 <= ii)      # block 2m+1 : (ZERO  | TRI_L)
    m3 = (jj >= ii - 128)      # block 2m-2 : (FULL  | TRI_U)
    m4 = (jj >= ii)            # block 2m-3 : (TRI_U | ZERO)
    m5 = m4 | m2               # merged slot: left half from block 2m-3,
    #                            right half from block 2m+1 (disjoint supports)
    return [m.astype(BF16_NP) for m in (m1, m2, m3, m5)]


def _extra_cells(pos0, pos1):
    """Random-attention cells not covered by global/window, deduped.
    Returns (I, J) row/col arrays (batch-independent)."""
    pos0 = np.asarray(pos0).astype(np.int64).ravel()
    pos1 = np.asarray(pos1).astype(np.int64).ravel()
    valid = pos0 > pos1
    p0, p1 = pos0[valid], pos1[valid]
    I = np.concatenate([p0, p0, p0 + 1, p0 + 1])
    J = np.concatenate([p1, p1 + 1, p1, p1 + 1])
    ok = (J <= I) & (I < L) & (J < L)
    covered = (J < GLOBAL) | (J >= I - WINDOW)
    keep = ok & ~covered
    I, J = I[keep], J[keep]
    lin = np.unique(I * L + J)
    return lin // L, lin % L


def _prepare(qkv, pos0, pos1):
    qkv = np.asarray(qkv, dtype=np.float32)
    q = qkv[:, :, 0:D]
    k = qkv[:, :, D : 2 * D]
    v = qkv[:, :, 2 * D : 3 * D]
    scale = 1.0 / float(np.sqrt(D))

    m1, m2, m3, m5 = _static_masks()

    I, J = _extra_cells(pos0, pos1)
    pair_of = I // PAIR_ROWS
    pair_cols = {}
    for m in range(L // PAIR_ROWS):
        sel = pair_of == m
        pair_cols[m] = np.unique(J[sel])
    e_max = max((len(c) for c in pair_cols.values()), default=0)
    NE = max(1, -(-e_max // 128))                          # extra slots per unit

    # per-pair data-independent extra masks [NE,128,256]
    pair_mx = {}
    for m, cols in pair_cols.items():
        mx = np.zeros((NE, 128, PAIR_ROWS), dtype=BF16_NP)
        if len(cols):
            sel = pair_of == m
            e = np.searchsorted(cols, J[sel])
            mx[e // 128, e % 128, I[sel] - m * PAIR_ROWS] = 1.0
        pair_mx[m] = mx

    in_maps = []
    for c in range(N_CORES):
        b, h = c // 2, c % 2
        kb, vb, qb = k[b], v[b], q[b]

        kvw = np.zeros((NWIN, 128, KVW_W), dtype=BF16_NP)
        for j in range(NWIN):
            blk = 16 * h - 3 + j
            if 2 <= blk < NB:                 # blocks 0,1 are served by the
                r0 = blk * BLOCK              # global slots; <0 don't exist
                kvw[j, :, : NCHUNK * 128] = _transpose_block(kb[r0 : r0 + BLOCK])
                kvw[j, :, NCHUNK * 128 : NCHUNK * 128 + D] = vb[r0 : r0 + BLOCK]
                kvw[j, :, NCHUNK * 128 + D] = 1.0

        ktg = np.concatenate(
            [_transpose_block(kb[0:BLOCK]), _transpose_block(kb[BLOCK:GLOBAL])], axis=1
        ).astype(BF16_NP)                                  # [128, 2*768]

        crest = np.zeros((128, CR_W), dtype=BF16_NP)       # m14 | vg
        for i, m in enumerate((m1, m2, m3, m5)):
            crest[:, i * PAIR_ROWS : (i + 1) * PAIR_ROWS] = m
        o = 4 * PAIR_ROWS
        crest[:, o : o + D] = vb[0:BLOCK]
        crest[:, o + D] = 1.0
        crest[:, o + DV : o + DV + D] = vb[BLOCK:GLOBAL]
        crest[:, o + DV + D] = 1.0

        qm = np.zeros((UNITS, 128, QM_W), dtype=BF16_NP)   # qt | mg
        kvx = np.zeros((UNITS * NE, 128, KVX_W), dtype=BF16_NP)
        for u in range(UNITS):
            m = 8 * h + u
            r0 = m * PAIR_ROWS
            qm[u, :, : NCHUNK * PAIR_ROWS] = _transpose_block(
                qb[r0 : r0 + PAIR_ROWS] * scale
            )
            og = NCHUNK * PAIR_ROWS
            qm[u, :, og:] = 1.0
            if m == 0:  # pair (0,1): global cols need the causal triangle
                qm[u, :, og : og + PAIR_ROWS] = m1
                qm[u, :, og + PAIR_ROWS :] = m2
            cols = pair_cols[m]
            for x in range(NE):
                cc = cols[x * 128 : (x + 1) * 128]
                row = kvx[u * NE + x]
                if len(cc):
                    kx = np.zeros((128, D), dtype=np.float32)
                    kx[: len(cc)] = kb[cc]
                    row[:, : NCHUNK * 128] = _transpose_block(kx)
                    row[: len(cc), NCHUNK * 128 : NCHUNK * 128 + D] = vb[cc]
                    row[: len(cc), NCHUNK * 128 + D] = 1.0
                row[:, NCHUNK * 128 + DV :] = pair_mx[m][x]

        in_maps.append({"qm": qm, "kvw": kvw, "ktg": ktg, "crest": crest, "kvx": kvx})
    return in_maps, NE


# ---------------------------------------------------------------------------
# device program (identical across cores; all variation is in the data)

def _build_program(NE):
    nc = bass.Bass()
    d_qm = nc.dram_tensor("qm", [UNITS, 128, QM_W], BF16, kind="ExternalInput")
    d_kvw = nc.dram_tensor("kvw", [NWIN, 128, KVW_W], BF16, kind="ExternalInput")
    d_ktg = nc.dram_tensor("ktg", [128, 2 * NCHUNK * 128], BF16, kind="ExternalInput")
    d_crest = nc.dram_tensor("crest", [128, CR_W], BF16, kind="ExternalInput")
    d_kvx = nc.dram_tensor("kvx", [UNITS * NE, 128, KVX_W], BF16, kind="ExternalInput")
    d_out = nc.dram_tensor("out", [UNITS, PAIR_ROWS, D], F32, kind="ExternalOutput")

    EXP = mybir.ActivationFunctionType.Exp
    KTW = NCHUNK * 128   # 768: kt width inside kvw/kvx rows

    with TileContext(nc) as tc:
        with (
            tc.tile_pool(name="const", bufs=1) as const_pool,
            tc.tile_pool(name="kvw", bufs=8) as kvw_pool,
            tc.tile_pool(name="qm", bufs=2) as qm_pool,
            tc.tile_pool(name="ext", bufs=NE + 1) as ext_pool,
            tc.tile_pool(name="pt", bufs=16) as pt_pool,
            tc.tile_pool(name="eps", bufs=4) as eps_pool,
            tc.tile_pool(name="ob", bufs=3) as ob_pool,
            tc.tile_pool(name="st", bufs=4, space="PSUM") as st_pool,
            tc.tile_pool(name="av", bufs=2, space="PSUM") as av_pool,
        ):
            # PE warm-up: HAM needs ~3.4us of activity to clock 1.2->2.4GHz.
            # Run throwaway matmuls on a zeroed tile while the first DMAs
            # are in flight so the real matmul stream starts warm.
            warm = const_pool.tile([128, 512], BF16)
            nc.vector.memset(warm[:], 0.0)
            warm_ps = st_pool.tile([128, 512], F32, tag="st", name="warm_ps")
            for _ in range(6):
                nc.tensor.matmul(
                    warm_ps[:], warm[:, 0:128], warm[:], start=True, stop=True
                )

            # first-matmul deps load first: ktg then qm[0]
            ktg_t = const_pool.tile([128, 2 * NCHUNK * 128], BF16)
            nc.sync.dma_start(out=ktg_t[:], in_=d_ktg[:])
            # split qt/mg halves so the first QK only waits on the qt bytes
            qm_first = qm_pool.tile([128, QM_W], BF16, tag="qm", name="qm0")
            _oq = NCHUNK * PAIR_ROWS
            nc.sync.dma_start(out=qm_first[:, :_oq], in_=d_qm[0, :, :_oq])
            nc.sync.dma_start(out=qm_first[:, _oq:], in_=d_qm[0, :, _oq:])

            kvw_tiles = {}

            def load_kvw(j):
                if j not in kvw_tiles:
                    t = kvw_pool.tile([128, KVW_W], BF16, tag="kvw", name=f"kvw{j}")
                    nc.sync.dma_start(out=t[:], in_=d_kvw[j])
                    kvw_tiles[j] = t

            load_kvw(1)
            load_kvw(2)
            crest_t = const_pool.tile([128, CR_W], BF16)
            nc.sync.dma_start(out=crest_t[:], in_=d_crest[:])

            def m14_ap(idx):
                return crest_t[:, idx * PAIR_ROWS : (idx + 1) * PAIR_ROWS]

            def vg_ap(idx):
                o = 4 * PAIR_ROWS + idx * DV
                return crest_t[:, o : o + DV]

            for u in range(UNITS):
                if u == 0:
                    qm_t = qm_first
                else:
                    qm_t = qm_pool.tile([128, QM_W], BF16, tag="qm", name=f"qm{u}")
                    nc.sync.dma_start(out=qm_t[:], in_=d_qm[u])
                qt = qm_t[:, : NCHUNK * PAIR_ROWS]

                def mg_ap(idx):
                    o = NCHUNK * PAIR_ROWS + idx * PAIR_ROWS
                    return qm_t[:, o : o + PAIR_ROWS]

                # slot order consumes cached tiles first; new tiles (2u+3,
                # 2u+4) load in the order the reordered slots need them
                for j in (2 * u + 1, 2 * u + 2, 2 * u + 3, 2 * u, 2 * u + 4):
                    load_kvw(j)
                kvx_ts = []
                for x in range(NE):
                    t = ext_pool.tile([128, KVX_W], BF16, tag="kvx", name=f"kvx{u}_{x}")
                    nc.sync.dma_start(out=t[:], in_=d_kvx[u * NE + x])
                    kvx_ts.append(t)

                def wkt(j):
                    return kvw_tiles[j][:, 0:KTW]

                def wv(j):
                    return kvw_tiles[j][:, KTW : KTW + DV]

                # slot list: (kT per half (same AP or two APs), V per half,
                # mask access or None). The merged slot computes the left
                # score half from block 2m-3 and the right half from block
                # 2m+1 (their masked supports are complementary halves).
                # mg is all-ones except pair 0 (h=0 cores' unit 0), whose
                # global columns need the causal triangle — all cores' unit 0
                # carries the multiply (h=1 data is ones), later units skip it
                slots = [
                    ((ktg_t[:, 0:KTW],) * 2, (vg_ap(0),) * 2,
                     mg_ap(0) if u == 0 else None),
                    ((ktg_t[:, KTW:],) * 2, (vg_ap(1),) * 2,
                     mg_ap(1) if u == 0 else None),
                    ((wkt(2 * u + 1),) * 2, (wv(2 * u + 1),) * 2, m14_ap(2)),  # M3
                    ((wkt(2 * u + 2),) * 2, (wv(2 * u + 2),) * 2, None),       # full
                    ((wkt(2 * u + 3),) * 2, (wv(2 * u + 3),) * 2, m14_ap(0)),  # M1
                    (
                        (wkt(2 * u), wkt(2 * u + 4)),
                        (wv(2 * u), wv(2 * u + 4)),
                        m14_ap(3),                           # M5 merged
                    ),
                ] + [
                    (
                        (kvx_ts[x][:, 0:KTW],) * 2,
                        (kvx_ts[x][:, KTW : KTW + DV],) * 2,
                        kvx_ts[x][:, KTW + DV :],
                    )
                    for x in range(NE)
                ]

                av = [
                    av_pool.tile([128, DV], F32, tag="av", name=f"av{u}_{hh}")
                    for hh in range(2)
                ]
                n_s = len(slots)

                def emit_av(si, pt, v_aps):
                    for hh in range(2):
                        lhs = pt[:, hh * 128 : (hh + 1) * 128]
                        v_ap = v_aps[hh]
                        nc.tensor.matmul(
                            av[hh][:, 0:512], lhs, v_ap[:, 0:512],
                            start=(si == 0), stop=(si == n_s - 1),
                        )
                        nc.tensor.matmul(
                            av[hh][:, 512:DV], lhs, v_ap[:, 512:DV],
                            start=(si == 0), stop=(si == n_s - 1),
                        )

                # AV for slot si is emitted after QK for slot si+SKEW so the
                # PE has guaranteed work while exp+mask of slot si complete.
                pending = []
                for si, (kt_aps, v_aps, m_ap) in enumerate(slots):
                    st = st_pool.tile([128, PAIR_ROWS], F32, tag="st", name=f"st{u}_{si}")
                    if kt_aps[0] is kt_aps[1]:
                        for cc in range(NCHUNK):
                            nc.tensor.matmul(
                                st[:],
                                kt_aps[0][:, cc * 128 : (cc + 1) * 128],
                                qt[:, cc * PAIR_ROWS : (cc + 1) * PAIR_ROWS],
                                start=(cc == 0),
                                stop=(cc == NCHUNK - 1),
                            )
                    else:
                        # two sequential accumulation groups in one PSUM bank
                        # (zero region = bank, so they must not interleave)
                        for hh in range(2):
                            for cc in range(NCHUNK):
                                nc.tensor.matmul(
                                    st[:, hh * 128 : (hh + 1) * 128],
                                    kt_aps[hh][:, cc * 128 : (cc + 1) * 128],
                                    qt[:, cc * PAIR_ROWS + hh * 128 : cc * PAIR_ROWS + hh * 128 + 128],
                                    start=(cc == 0),
                                    stop=(cc == NCHUNK - 1),
                                )
                    pt = pt_pool.tile([128, PAIR_ROWS], BF16, tag="pt", name=f"pt{u}_{si}")
                    nc.scalar.activation(pt[:], st[:], EXP)
                    if m_ap is not None:
                        nc.vector.tensor_mul(pt[:], pt[:], m_ap)
                    pending.append((si, pt, v_aps))
                    if len(pending) > 2:
                        emit_av(*pending.pop(0))
                for p in pending:
                    emit_av(*p)
                for hh in range(2):
                    rc = eps_pool.tile([128, 1], F32, tag="rc", name=f"rc{u}_{hh}")
                    nc.vector.reciprocal(rc[:], av[hh][:, D : D + 1])
                    ob = ob_pool.tile([128, D], F32, tag="ob", name=f"ob{u}_{hh}")
                    # chunked scale+store, spread across engines (scale on
                    # ACT for half 0 / DVE for half 1; store triggers on
                    # GpSimd / Sync) so the kernel tail pipelines
                    for lo, hi in ((0, 384), (384, D)):
                        if hh == 0:
                            nc.scalar.mul(ob[:, lo:hi], av[hh][:, lo:hi], rc[:])
                            st_eng = nc.gpsimd
                        else:
                            nc.vector.tensor_scalar_mul(
                                ob[:, lo:hi], av[hh][:, lo:hi], rc[:]
                            )
                            st_eng = nc.sync
                        st_eng.dma_start(
                            out=d_out[u, hh * 128 : (hh + 1) * 128, lo:hi],
                            in_=ob[:, lo:hi],
                        )

    _split_excess_waits(nc, max_w=1)
    return nc


_program_cache = {}


def kernel(qkv, pos0, pos1):
    global _last_results
    in_maps, NE = _prepare(qkv, pos0, pos1)
    if NE not in _program_cache:
        _program_cache[NE] = _build_program(NE)
    nc = _program_cache[NE]
    res = run_bass_kernel_spmd(
        nc, in_maps, core_ids=list(range(N_CORES)),
        trace=bool(os.environ.get("BASS_TRACE")),
    )
    _last_results = res
    out = np.empty((B, L, D), dtype=np.float32)
    for c in range(N_CORES):
        b, h = c // 2, c % 2
        for u in range(UNITS):
            r0 = (8 * h + u) * PAIR_ROWS
            out[b, r0 : r0 + PAIR_ROWS, :] = res.results[c]["out"][u]
    return out


# revision 42
# speedup vs baseline: 1.0239x; 1.0135x over previous
"""BigBird block-sparse self-attention on 8 Trainium2 NeuronCores.

Reference semantics (B=4, l=4096, d=768, BLOCK=128):
  q,k,v = split(qkv); scores = q @ k^T / sqrt(d)
  mask: row i attends to j iff j<=i AND (j < 256 [global cols]
        OR j >= i-384 [sliding window] OR (i,j) in random 2x2 cells
        derived from pos0/pos1 pairs with pos0>pos1)
  out = softmax(scores + mask) @ v   (masked entries get <=-1e4, whose
        exp underflows to exactly 0 in fp32)

Strategy: 128-row blocks paired into 256-row "pair" units (N=256 matmul
free dim). 16 pairs x 4 batches = 64 units; 8 cores x 8 units each
(core c: batch c//2, pairs 8*(c%2)..8*(c%2)+7). Per unit the column
blocks are: 2 global blocks, 5 window blocks (2m-3..2m+1), and NE
gathered "extra" blocks holding the random cells not covered by
global/window. Scores are computed transposed (S^T[jj,ii]) so softmax
normalization comes out of the AV matmul via a ones-column appended to
V; masks are 0/1 multiplicative on P=exp(S^T) (underflow-equivalent to
the reference's additive -1e4). Matmuls run in bf16 with fp32 PSUM
accumulation; output is fp32.

K and V for each column block are packed into one HBM tensor so each
block loads with a single DMA (HWDGE trigger costs ~0.6us of Sync
sequencer time); output stores go through GpSimd SWDGE to keep the
Sync sequencer off the critical path.
"""

import os
import sys

for _p in ("/opt/trn_rl_repo", "/root/.axon_site/_ro/trn_rl_repo"):
    if _p not in sys.path and os.path.isdir(_p):
        sys.path.append(_p)

import ml_dtypes
import numpy as np

import concourse.bass as bass
import concourse.mybir as mybir
from bass_rust import InstNoOp
from concourse import tile as tile_mod
from concourse.bass_utils import run_bass_kernel_spmd
from concourse.tile import TileContext


def _fast_exit(self, tick_clock, wait_clock):
    """Cheaper TileContext exit than drain + EVSEM butterfly x2 (~8-10us).

    The SP drain already carries sem waits covering the global vector clock
    (= every engine's and DMA queue's final tick), so once it retires no
    engine is running or waiting. A single cross-engine sem then orders the
    GpSimd sem/dma-queue clears after it. Saves ~6us of kernel tail.
    """
    nc = self.nc
    drain_inst = nc.sync.drain()
    wait_clock.add_sem_waits(
        drain_inst.ins, tile_mod.ScopedClock({None: tick_clock.global_clock})
    )
    # split the global-clock wait chain across SP and GpSimd so the two
    # halves are checked in parallel (the post-pass later explodes each
    # side into 1-wait NoOps for walrus)
    si = drain_inst.ins.sync_info
    waits = list(si.on_wait) if (si is not None and si.on_wait) else []
    mid = len(waits) // 2
    if mid:
        drain_inst.ins.sync_info = mybir.SyncInfo(
            on_wait=waits[mid:], on_update=list(si.on_update or [])
        )
    done = nc.alloc_semaphore("tile_exit_done")
    drain_inst.then_inc(done)
    popped = nc._tile_sem_poison_stack.pop()
    assert popped is self._sem_poison
    if mid:
        gn = nc.gpsimd.nop()
        gn.ins.sync_info = mybir.SyncInfo(on_wait=waits[:mid], on_update=[])
    nc.gpsimd.wait_ge(done, 1)
    nc.clear_and_free_semaphores(list(self.sems.allocated().values()) + [done])


tile_mod.TileContext._drain_and_barrier = _fast_exit

BLOCK = 128
B, L, D = 4, 4096, 768
NB = L // BLOCK          # 32 column blocks
WINDOW = 3 * BLOCK       # 384
GLOBAL = 2 * BLOCK       # 256
NCHUNK = D // 128        # 6 contraction chunks
N_CORES = 8
UNITS = 8                # pairs per core
PAIR_ROWS = 2 * BLOCK    # 256
NWIN = 2 * UNITS + 3     # 19 window blocks cached per core
DV = D + 1               # v block width incl. ones column (769)

F32 = mybir.dt.float32
BF16 = mybir.dt.bfloat16
BF16_NP = ml_dtypes.bfloat16

# packed free-dim offsets
QM_W = NCHUNK * PAIR_ROWS + 2 * PAIR_ROWS          # qt | mg      (2048)
KVW_W = NCHUNK * 128 + DV                          # kt | v       (1537)
KVX_W = NCHUNK * 128 + DV + PAIR_ROWS              # kt | v | mx  (1793)
CR_W = 4 * PAIR_ROWS + 2 * DV                      # m14 | vg     (2562)

_last_results = None     # test harness reads exec_time_ns from here


# ---------------------------------------------------------------------------
# walrus in this toolchain rejects >1 sync-wait command per instruction;
# split excess waits onto same-engine NoOps inserted just before.
def _split_excess_waits(nc, max_w=1):
    ctr = 0
    for blk in nc.m.functions[0].blocks:
        out = []
        changed = False
        for inst in blk.instructions:
            si = inst.sync_info
            waits = list(si.on_wait) if (si is not None and si.on_wait) else []
            if len(waits) > max_w:
                changed = True
                excess, keep = waits[:-max_w], waits[-max_w:]
                for i in range(0, len(excess), max_w):
                    ctr += 1
                    nop = InstNoOp(name=f"wsplit_{ctr}", ins=[], outs=[])
                    nop.engine = inst.engine
                    nop.sync_info = mybir.SyncInfo(
                        on_wait=excess[i : i + max_w], on_update=[]
                    )
                    out.append(nop)
                inst.sync_info = mybir.SyncInfo(
                    on_wait=keep, on_update=list(si.on_update or [])
                )
            out.append(inst)
        if changed:
            blk.instructions = out


# ---------------------------------------------------------------------------
# host-side data prep

def _transpose_block(x):
    """[rows, 768] -> [128, NCHUNK*rows] with [p, c*rows+r] = x[r, c*128+p]."""
    rows = x.shape[0]
    return np.ascontiguousarray(
        x.reshape(rows, NCHUNK, 128).transpose(2, 1, 0).reshape(128, NCHUNK * rows)
    )


def _static_masks():
    ii = np.arange(PAIR_ROWS)[None, :]
    jj = np.arange(128)[:, None]
    m1 = (jj <= ii)            # block 2m   : (TRI_L | FULL)
    m2 = (jj + 128 <= ii)      # block 2m+1 : (ZERO  | TRI_L)
    m3 = (jj >= ii - 128)      # block 2m-2 : (FULL  | TRI_U)
    m4 = (jj >= ii)            # block 2m-3 : (TRI_U | ZERO)
    m5 = m4 | m2               # merged slot: left half from block 2m-3,
    #                            right half from block 2m+1 (disjoint supports)
    return [m.astype(BF16_NP) for m in (m1, m2, m3, m5)]


def _extra_cells(pos0, pos1):
    """Random-attention cells not covered by global/window, deduped.
    Returns (I, J) row/col arrays (batch-independent)."""
    pos0 = np.asarray(pos0).astype(np.int64).ravel()
    pos1 = np.asarray(pos1).astype(np.int64).ravel()
    valid = pos0 > pos1
    p0, p1 = pos0[valid], pos1[valid]
    I = np.concatenate([p0, p0, p0 + 1, p0 + 1])
    J = np.concatenate([p1, p1 + 1, p1, p1 + 1])
    ok = (J <= I) & (I < L) & (J < L)
    covered = (J < GLOBAL) | (J >= I - WINDOW)
    keep = ok & ~covered
    I, J = I[keep], J[keep]
    lin = np.unique(I * L + J)
    return lin // L, lin % L


def _prepare(qkv, pos0, pos1):
    qkv = np.asarray(qkv, dtype=np.float32)
    q = qkv[:, :, 0:D]
    k = qkv[:, :, D : 2 * D]
    v = qkv[:, :, 2 * D : 3 * D]
    scale = 1.0 / float(np.sqrt(D))

    m1, m2, m3, m5 = _static_masks()

    I, J = _extra_cells(pos0, pos1)
    pair_of = I // PAIR_ROWS
    pair_cols = {}
    for m in range(L // PAIR_ROWS):
        sel = pair_of == m
        pair_cols[m] = np.unique(J[sel])
    e_max = max((len(c) for c in pair_cols.values()), default=0)
    NE = max(1, -(-e_max // 128))                          # extra slots per unit

    # per-pair data-independent extra masks [NE,128,256]
    pair_mx = {}
    for m, cols in pair_cols.items():
        mx = np.zeros((NE, 128, PAIR_ROWS), dtype=BF16_NP)
        if len(cols):
            sel = pair_of == m
            e = np.searchsorted(cols, J[sel])
            mx[e // 128, e % 128, I[sel] - m * PAIR_ROWS] = 1.0
        pair_mx[m] = mx

    in_maps = []
    for c in range(N_CORES):
        b, h = c // 2, c % 2
        kb, vb, qb = k[b], v[b], q[b]

        kvw = np.zeros((NWIN, 128, KVW_W), dtype=BF16_NP)
        for j in range(NWIN):
            blk = 16 * h - 3 + j
            if 2 <= blk < NB:                 # blocks 0,1 are served by the
                r0 = blk * BLOCK              # global slots; <0 don't exist
                kvw[j, :, : NCHUNK * 128] = _transpose_block(kb[r0 : r0 + BLOCK])
                kvw[j, :, NCHUNK * 128 : NCHUNK * 128 + D] = vb[r0 : r0 + BLOCK]
                kvw[j, :, NCHUNK * 128 + D] = 1.0

        ktg = np.concatenate(
            [_transpose_block(kb[0:BLOCK]), _transpose_block(kb[BLOCK:GLOBAL])], axis=1
        ).astype(BF16_NP)                                  # [128, 2*768]

        crest = np.zeros((128, CR_W), dtype=BF16_NP)       # m14 | vg
        for i, m in enumerate((m1, m2, m3, m5)):
            crest[:, i * PAIR_ROWS : (i + 1) * PAIR_ROWS] = m
        o = 4 * PAIR_ROWS
        crest[:, o : o + D] = vb[0:BLOCK]
        crest[:, o + D] = 1.0
        crest[:, o + DV : o + DV + D] = vb[BLOCK:GLOBAL]
        crest[:, o + DV + D] = 1.0

        qm = np.zeros((UNITS, 128, QM_W), dtype=BF16_NP)   # qt | mg
        kvx = np.zeros((UNITS * NE, 128, KVX_W), dtype=BF16_NP)
        for u in range(UNITS):
            m = 8 * h + u
            r0 = m * PAIR_ROWS
            qm[u, :, : NCHUNK * PAIR_ROWS] = _transpose_block(
                qb[r0 : r0 + PAIR_ROWS] * scale
            )
            og = NCHUNK * PAIR_ROWS
            qm[u, :, og:] = 1.0
            if m == 0:  # pair (0,1): global cols need the causal triangle
                qm[u, :, og : og + PAIR_ROWS] = m1
                qm[u, :, og + PAIR_ROWS :] = m2
            cols = pair_cols[m]
            for x in range(NE):
                cc = cols[x * 128 : (x + 1) * 128]
                row = kvx[u * NE + x]
                if len(cc):
                    kx = np.zeros((128, D), dtype=np.float32)
                    kx[: len(cc)] = kb[cc]
                    row[:, : NCHUNK * 128] = _transpose_block(kx)
                    row[: len(cc), NCHUNK * 128 : NCHUNK * 128 + D] = vb[cc]
                    row[: len(cc), NCHUNK * 128 + D] = 1.0
                row[:, NCHUNK * 128 + DV :] = pair_mx[m][x]

        in_maps.append({"qm": qm, "kvw": kvw, "ktg": ktg, "crest": crest, "kvx": kvx})
    return in_maps, NE


# ---------------------------------------------------------------------------
# device program (identical across cores; all variation is in the data)

def _build_program(NE):
    nc = bass.Bass()
    d_qm = nc.dram_tensor("qm", [UNITS, 128, QM_W], BF16, kind="ExternalInput")
    d_kvw = nc.dram_tensor("kvw", [NWIN, 128, KVW_W], BF16, kind="ExternalInput")
    d_ktg = nc.dram_tensor("ktg", [128, 2 * NCHUNK * 128], BF16, kind="ExternalInput")
    d_crest = nc.dram_tensor("crest", [128, CR_W], BF16, kind="ExternalInput")
    d_kvx = nc.dram_tensor("kvx", [UNITS * NE, 128, KVX_W], BF16, kind="ExternalInput")
    d_out = nc.dram_tensor("out", [UNITS, PAIR_ROWS, D], F32, kind="ExternalOutput")

    EXP = mybir.ActivationFunctionType.Exp
    KTW = NCHUNK * 128   # 768: kt width inside kvw/kvx rows

    with TileContext(nc) as tc:
        with (
            tc.tile_pool(name="const", bufs=1) as const_pool,
            tc.tile_pool(name="kvw", bufs=12) as kvw_pool,
            tc.tile_pool(name="qm", bufs=3) as qm_pool,
            tc.tile_pool(name="ext", bufs=NE + 2) as ext_pool,
            tc.tile_pool(name="pt", bufs=16) as pt_pool,
            tc.tile_pool(name="eps", bufs=4) as eps_pool,
            tc.tile_pool(name="ob", bufs=4) as ob_pool,
            tc.tile_pool(name="st", bufs=4, space="PSUM") as st_pool,
            tc.tile_pool(name="av", bufs=2, space="PSUM") as av_pool,
        ):
            # PE warm-up: HAM needs ~3.4us of activity to clock 1.2->2.4GHz.
            # Run throwaway matmuls on a zeroed tile while the first DMAs
            # are in flight so the real matmul stream starts warm.
            warm = const_pool.tile([128, 512], BF16)
            nc.vector.memset(warm[:], 0.0)
            warm_ps = st_pool.tile([128, 512], F32, tag="st", name="warm_ps")
            for _ in range(6):
                nc.tensor.matmul(
                    warm_ps[:], warm[:, 0:128], warm[:], start=True, stop=True
                )

            # first-matmul deps load first: ktg then qm[0]
            ktg_t = const_pool.tile([128, 2 * NCHUNK * 128], BF16)
            nc.sync.dma_start(out=ktg_t[:], in_=d_ktg[:])
            # split qt/mg halves so the first QK only waits on the qt bytes
            qm_first = qm_pool.tile([128, QM_W], BF16, tag="qm", name="qm0")
            _oq = NCHUNK * PAIR_ROWS
            nc.sync.dma_start(out=qm_first[:, :_oq], in_=d_qm[0, :, :_oq])
            nc.sync.dma_start(out=qm_first[:, _oq:], in_=d_qm[0, :, _oq:])

            kvw_tiles = {}

            def load_kvw(j):
                if j not in kvw_tiles:
                    t = kvw_pool.tile([128, KVW_W], BF16, tag="kvw", name=f"kvw{j}")
                    nc.sync.dma_start(out=t[:], in_=d_kvw[j])
                    kvw_tiles[j] = t

            load_kvw(1)
            load_kvw(2)
            crest_t = const_pool.tile([128, CR_W], BF16)
            nc.sync.dma_start(out=crest_t[:], in_=d_crest[:])

            def m14_ap(idx):
                return crest_t[:, idx * PAIR_ROWS : (idx + 1) * PAIR_ROWS]

            def vg_ap(idx):
                o = 4 * PAIR_ROWS + idx * DV
                return crest_t[:, o : o + DV]

            for u in range(UNITS):
                if u == 0:
                    qm_t = qm_first
                else:
                    qm_t = qm_pool.tile([128, QM_W], BF16, tag="qm", name=f"qm{u}")
                    nc.sync.dma_start(out=qm_t[:], in_=d_qm[u])
                qt = qm_t[:, : NCHUNK * PAIR_ROWS]

                def mg_ap(idx):
                    o = NCHUNK * PAIR_ROWS + idx * PAIR_ROWS
                    return qm_t[:, o : o + PAIR_ROWS]

                # slot order consumes cached tiles first; new tiles (2u+3,
                # 2u+4) load in the order the reordered slots need them
                for j in (2 * u + 1, 2 * u + 2, 2 * u + 3, 2 * u, 2 * u + 4):
                    load_kvw(j)
                kvx_ts = []
                for x in range(NE):
                    t = ext_pool.tile([128, KVX_W], BF16, tag="kvx", name=f"kvx{u}_{x}")
                    nc.sync.dma_start(out=t[:], in_=d_kvx[u * NE + x])
                    kvx_ts.append(t)

                def wkt(j):
                    return kvw_tiles[j][:, 0:KTW]

                def wv(j):
                    return kvw_tiles[j][:, KTW : KTW + DV]

                # slot list: (kT per half (same AP or two APs), V per half,
                # mask access or None). The merged slot computes the left
                # score half from block 2m-3 and the right half from block
                # 2m+1 (their masked supports are complementary halves).
                # mg is all-ones except pair 0 (h=0 cores' unit 0), whose
                # global columns need the causal triangle — all cores' unit 0
                # carries the multiply (h=1 data is ones), later units skip it
                slots = [
                    ((ktg_t[:, 0:KTW],) * 2, (vg_ap(0),) * 2,
                     mg_ap(0) if u == 0 else None),
                    ((ktg_t[:, KTW:],) * 2, (vg_ap(1),) * 2,
                     mg_ap(1) if u == 0 else None),
                    ((wkt(2 * u + 1),) * 2, (wv(2 * u + 1),) * 2, m14_ap(2)),  # M3
                    ((wkt(2 * u + 2),) * 2, (wv(2 * u + 2),) * 2, None),       # full
                    ((wkt(2 * u + 3),) * 2, (wv(2 * u + 3),) * 2, m14_ap(0)),  # M1
                    (
                        (wkt(2 * u), wkt(2 * u + 4)),
                        (wv(2 * u), wv(2 * u + 4)),
                        m14_ap(3),                           # M5 merged
                    ),
                ] + [
                    (
                        (kvx_ts[x][:, 0:KTW],) * 2,
                        (kvx_ts[x][:, KTW : KTW + DV],) * 2,
                        kvx_ts[x][:, KTW + DV :],
                    )
                    for x in range(NE)
                ]

                av = [
                    av_pool.tile([128, DV], F32, tag="av", name=f"av{u}_{hh}")
                    for hh in range(2)
                ]
                n_s = len(slots)

                def emit_av(si, pt, v_aps):
                    for hh in range(2):
                        lhs = pt[:, hh * 128 : (hh + 1) * 128]
                        v_ap = v_aps[hh]
                        nc.tensor.matmul(
                            av[hh][:, 0:512], lhs, v_ap[:, 0:512],
                            start=(si == 0), stop=(si == n_s - 1),
                        )
                        nc.tensor.matmul(
                            av[hh][:, 512:DV], lhs, v_ap[:, 512:DV],
                            start=(si == 0), stop=(si == n_s - 1),
                        )

                # AV for slot si is emitted after QK for slot si+SKEW so the
                # PE has guaranteed work while exp+mask of slot si complete.
                pending = []
                for si, (kt_aps, v_aps, m_ap) in enumerate(slots):
                    st = st_pool.tile([128, PAIR_ROWS], F32, tag="st", name=f"st{u}_{si}")
                    if kt_aps[0] is kt_aps[1]:
                        for cc in range(NCHUNK):
                            nc.tensor.matmul(
                                st[:],
                                kt_aps[0][:, cc * 128 : (cc + 1) * 128],
                                qt[:, cc * PAIR_ROWS : (cc + 1) * PAIR_ROWS],
                                start=(cc == 0),
                                stop=(cc == NCHUNK - 1),
                            )
                    else:
                        # two sequential accumulation groups in one PSUM bank
                        # (zero region = bank, so they must not interleave)
                        for hh in range(2):
                            for cc in range(NCHUNK):
                                nc.tensor.matmul(
                                    st[:, hh * 128 : (hh + 1) * 128],
                                    kt_aps[hh][:, cc * 128 : (cc + 1) * 128],
                                    qt[:, cc * PAIR_ROWS + hh * 128 : cc * PAIR_ROWS + hh * 128 + 128],
                                    start=(cc == 0),
                                    stop=(cc == NCHUNK - 1),
                                )
                    pt = pt_pool.tile([128, PAIR_ROWS], BF16, tag="pt", name=f"pt{u}_{si}")
                    nc.scalar.activation(pt[:], st[:], EXP)
                    if m_ap is not None:
                        nc.vector.tensor_mul(pt[:], pt[:], m_ap)
                    pending.append((si, pt, v_aps))
                    if len(pending) > 2:
                        emit_av(*pending.pop(0))
                for p in pending:
                    emit_av(*p)
                for hh in range(2):
                    rc = eps_pool.tile([128, 1], F32, tag="rc", name=f"rc{u}_{hh}")
                    nc.vector.reciprocal(rc[:], av[hh][:, D : D + 1])
                    ob = ob_pool.tile([128, D], F32, tag="ob", name=f"ob{u}_{hh}")
                    # chunked scale+store, spread across engines (scale on
                    # ACT for half 0 / DVE for half 1; store triggers on
                    # GpSimd / Sync) so the kernel tail pipelines
                    for lo, hi in ((0, 384), (384, D)):
                        if hh == 0:
                            nc.scalar.mul(ob[:, lo:hi], av[hh][:, lo:hi], rc[:])
                            st_eng = nc.gpsimd
                        else:
                            nc.vector.tensor_scalar_mul(
                                ob[:, lo:hi], av[hh][:, lo:hi], rc[:]
                            )
                            st_eng = nc.sync
                        st_eng.dma_start(
                            out=d_out[u, hh * 128 : (hh + 1) * 128, lo:hi],
                            in_=ob[:, lo:hi],
                        )

    _split_excess_waits(nc, max_w=1)
    return nc


_program_cache = {}


def kernel(qkv, pos0, pos1):
    global _last_results
    in_maps, NE = _prepare(qkv, pos0, pos1)
    if NE not in _program_cache:
        _program_cache[NE] = _build_program(NE)
    nc = _program_cache[NE]
    res = run_bass_kernel_spmd(
        nc, in_maps, core_ids=list(range(N_CORES)),
        trace=bool(os.environ.get("BASS_TRACE")),
    )
    _last_results = res
    out = np.empty((B, L, D), dtype=np.float32)
    for c in range(N_CORES):
        b, h = c // 2, c % 2
        for u in range(UNITS):
            r0 = (8 * h + u) * PAIR_ROWS
            out[b, r0 : r0 + PAIR_ROWS, :] = res.results[c]["out"][u]
    return out
